# revision 11
# baseline (speedup 1.0000x reference)
"""Trainium2 Bass kernel for the kinematic bicycle-model rollout.

Strategy
--------
The recurrence is affine in the start state with batch-independent
coefficients, so the host precomputes (float64) the [H] vectors
    c[t]  = sum_{i<t} DT*MAX_ACC*clip(a_i)          (speed_t = s0 + c_t)
    A[t]  = sum_{i<t} k_i,  Bv[t] = sum_{i<t} c_i*k_i,
            k_i = tan(clip(s_i))/WHEELBASE*DT       (yaw_t = yaw0 + s0*A_t + Bv_t)
leaving on-chip:
    x_t = x0 + sum_{i<t} (DT*c_i + DT*s0) * cos(yaw_i)   (same for y with sin)

Per core (data-parallel over batch, 1024 rollouts/core), batch on the 128
SBUF partitions (8 tiles), time on the free dim (2048). The 33.5 MB/core
of output DMA (~108 us at ~320 GB/s) is the roofline; work is spread
across all four compute engines to approach it:
  - PE:   yaw = s0*A + Bv + yaw0 (K=3 fp32 matmul -> PSUM, ~69us) and
          speed = s0 + c (K=2 bf16 matmul, ~8us; bf16 keeps speed's
          rel err ~5e-3, well under the 2e-2 gate, and speed feeds
          nothing else)
  - ACT:  yaw and speed PSUM->SBUF copies; the magic-constant round
          t1a = u+MAGIC, t1b = MAGIC-t1a = -round(u) (u = yaw/2pi);
          sin = Sin(2pi*w) and cos = Sin(pi/2-2pi|w|) written as fp16
  - DVE:  w = u - round(u) (stt reading PSUM); vx = cdts0*cos (fp16
          tensor_tensor -> 2x packed mode); x/y prefix sums via
          tensor_tensor_scan (fp32 state, fp16 inputs)
  - GpSimd: cdts0 = DT*c + DT*s0 (stride-0 broadcast add) and
          vy = cdts0*sin, both fp16 tensor_tensor on the Q7 cores
Outputs are written batch-major [1024, 2048]; the host transposes to the
reference's [H, B] layout after gathering the 8 shards.
"""

import math
import sys

sys.path.insert(0, "/opt/trn_rl_repo")

import numpy as np

import concourse.bacc as bacc
import concourse.mybir as mybir
import concourse.tile as tile
from concourse.bass_utils import run_bass_kernel_spmd

# Model constants (match the reference nn.Module)
H = 2048
B = 8192
NCORES = 8
BL = B // NCORES          # batch per core
P = 128                   # SBUF partitions
NPT = BL // P             # batch tiles per core
DT = 0.05
WHEELBASE = 2.5
MAX_STEER = 0.5
MAX_ACC = 5000.0 / 1000.0

TWO_PI = 2.0 * math.pi
INV_2PI = 1.0 / TWO_PI
HALF_PI = 0.5 * math.pi
MAGIC = 12582912.0        # 1.5 * 2**23: x + MAGIC - MAGIC == round(x) in f32

F32 = mybir.dt.float32
F32R = mybir.dt.float32r
F16 = mybir.dt.float16
BF16 = mybir.dt.bfloat16
AFT = mybir.ActivationFunctionType
ALU = mybir.AluOpType

_CACHE = {}


def _build():
    nc = bacc.Bacc("TRN2", target_bir_lowering=False, debug=False)

    rhs_yaw = nc.declare_dram_parameter("rhs_yaw", [3, H], F32R, isOutput=False)
    rhs_t1 = nc.declare_dram_parameter("rhs_t1", [4, H], BF16, isOutput=False)
    cdtrow = nc.declare_dram_parameter("cdtrow", [H], F16, isOutput=False)
    crow = nc.declare_dram_parameter("crow", [H], F32, isOutput=False)
    lhs3r = nc.declare_dram_parameter("lhs3r", [3, BL], F32R, isOutput=False)
    lhs4r = nc.declare_dram_parameter("lhs4r", [4, BL], BF16, isOutput=False)
    cols = nc.declare_dram_parameter("cols", [BL, 4], F32, isOutput=False)
    sdth = nc.declare_dram_parameter("sdth", [P, NPT], F16, isOutput=False)
    ox = nc.declare_dram_parameter("ox", [BL, H], F32, isOutput=True)
    oy = nc.declare_dram_parameter("oy", [BL, H], F32, isOutput=True)
    oyaw = nc.declare_dram_parameter("oyaw", [BL, H], F32, isOutput=True)
    ospeed = nc.declare_dram_parameter("ospeed", [BL, H], F32, isOutput=True)

    with tile.TileContext(nc) as tc:
        with (
            tc.tile_pool(name="const", bufs=1) as constp,
            tc.tile_pool(name="io", bufs=3) as iop,
            tc.tile_pool(name="mid", bufs=2) as midp,
            tc.tile_pool(name="psum", bufs=2, space="PSUM") as psp,
            tc.tile_pool(name="cpool", bufs=1) as cdp,
        ):
            cdt_bc = constp.tile([P, H], F16)
            nc.sync.dma_start(out=cdt_bc[:],
                              in_=cdtrow[None, :].to_broadcast((P, H)))
            rhs_yaw_sb = constp.tile([3, H], F32R)
            nc.sync.dma_start(out=rhs_yaw_sb[:], in_=rhs_yaw[:])
            rhs_t1_sb = constp.tile([4, H], BF16)
            nc.sync.dma_start(out=rhs_t1_sb[:], in_=rhs_t1[:])
            c_bc = constp.tile([P, H], F32)
            nc.sync.dma_start(out=c_bc[:],
                              in_=crow[None, :].to_broadcast((P, H)))
            lhs3 = constp.tile([3, BL], F32R)
            nc.sync.dma_start(out=lhs3[:], in_=lhs3r[:])
            lhs4 = constp.tile([4, BL], BF16)
            nc.sync.dma_start(out=lhs4[:], in_=lhs4r[:])
            halfpi_col = constp.tile([P, 1], F32)
            nc.vector.memset(halfpi_col[:], HALF_PI)
            negmagic_col = constp.tile([P, 1], F32)
            nc.vector.memset(negmagic_col[:], -MAGIC)

            # all cdts0 tiles upfront: the Q7's slow semaphore path is
            # hidden behind the first tiles' compute
            sdt_all = constp.tile([P, NPT], F16)
            nc.sync.dma_start(out=sdt_all[:], in_=sdth[:])
            cdts0_all = []
            for pt in range(NPT):
                ct = cdp.tile([P, H], F16, tag=f"cdts0_{pt}")
                nc.gpsimd.tensor_tensor(
                    out=ct[:], in0=cdt_bc[:],
                    in1=sdt_all[:, pt : pt + 1].to_broadcast((P, H)),
                    op=ALU.add,
                )
                cdts0_all.append(ct)

            for pt in range(NPT):
                sl = slice(pt * P, (pt + 1) * P)
                colt = iop.tile([P, 4], F32, tag="colt")
                nc.sync.dma_start(out=colt[:], in_=cols[sl, :])
                x0_c = colt[:, 0:1]
                y0_c = colt[:, 1:2]
                s0_c = colt[:, 2:3]
                cdts0 = cdts0_all[pt]

                x_sb = iop.tile([P, H], F32, tag="x")
                y_sb = iop.tile([P, H], F32, tag="y")
                yaw_sb = iop.tile([P, H], F32, tag="yaw")
                speed_sb = iop.tile([P, H], F32, tag="speed")
                nc.scalar.activation(out=x_sb[:, 0:1], in_=x0_c, func=AFT.Copy)
                nc.scalar.activation(out=y_sb[:, 0:1], in_=y0_c, func=AFT.Copy)

                NSEG = 2
                HH = H // NSEG
                for hf in range(NSEG):
                    cs = slice(hf * HH, (hf + 1) * HH)
                    # yaw = s0*A + Bv + yaw0 (radians);
                    # t1 = MAGIC - u, u = yaw/2pi (bf16 K=4, MAGIC row LAST
                    # so the fp32 PSUM row-order accumulation rounds u)
                    yaw_ps = psp.tile([P, HH], F32, tag="yawps")
                    t1_ps = psp.tile([P, HH], F32, tag="t1ps")
                    for j in range(HH // 512):
                        js = slice(j * 512, (j + 1) * 512)
                        rs = slice(hf * HH + j * 512, hf * HH + (j + 1) * 512)
                        nc.tensor.matmul(yaw_ps[:, js], lhs3[:, sl],
                                         rhs_yaw_sb[:, rs])
                        nc.tensor.matmul(t1_ps[:, js], lhs4[:, sl],
                                         rhs_t1_sb[:, rs])

                    nc.scalar.activation(out=yaw_sb[:, cs], in_=yaw_ps[:],
                                         func=AFT.Copy)
                    # speed = c + s0
                    nc.scalar.activation(out=speed_sb[:, cs], in_=c_bc[:, cs],
                                         func=AFT.Identity, bias=s0_c)

                    # rnn = t1 - MAGIC = -round(u); exact (Sterbenz)
                    rnn = midp.tile([P, HH], F32, tag="rnn")
                    nc.scalar.activation(out=rnn[:], in_=t1_ps[:],
                                         func=AFT.Identity,
                                         bias=negmagic_col[:])
                    # w = yaw/2pi - round(u)  in ~[-.58, .58]
                    w = midp.tile([P, HH], F32, tag="w")
                    nc.vector.scalar_tensor_tensor(
                        out=w[:], in0=yaw_ps[:], scalar=INV_2PI, in1=rnn[:],
                        op0=ALU.mult, op1=ALU.add,
                    )

                    # sin/cos via the Sin LUT (valid on [-pi, pi]), fp16 out
                    sin_t = midp.tile([P, HH], F16, tag="sin")
                    nc.scalar.activation(out=sin_t[:], in_=w[:], func=AFT.Sin,
                                         scale=TWO_PI)
                    nc.scalar.activation(out=w[:], in_=w[:], func=AFT.Abs)
                    cos_t = midp.tile([P, HH], F16, tag="cos")
                    nc.scalar.activation(
                        out=cos_t[:], in_=w[:], func=AFT.Sin, scale=-TWO_PI,
                        bias=halfpi_col[:],
                    )

                    # vx/vy on DVE (fp16 2x packed)
                    vx = midp.tile([P, HH], F16, tag="vx")
                    nc.vector.tensor_tensor(out=vx[:], in0=cdts0[:, cs],
                                            in1=cos_t[:], op=ALU.mult)
                    vy = midp.tile([P, HH], F16, tag="vy")
                    nc.vector.tensor_tensor(out=vy[:], in0=cdts0[:, cs],
                                            in1=sin_t[:], op=ALU.mult)

                    # seg scans chained off x[s*HH]; last seg drops v[H-1]
                    lo = hf * HH
                    nd = HH if hf < NSEG - 1 else HH - 1
                    nc.vector.tensor_tensor_scan(
                        out=x_sb[:, lo + 1 : lo + 1 + nd], data0=vx[:, 0:nd],
                        data1=vx[:, 0:nd],
                        initial=(x0_c if hf == 0 else x_sb[:, lo : lo + 1]),
                        op0=ALU.add, op1=ALU.bypass,
                    )
                    nc.vector.tensor_tensor_scan(
                        out=y_sb[:, lo + 1 : lo + 1 + nd], data0=vy[:, 0:nd],
                        data1=vy[:, 0:nd],
                        initial=(y0_c if hf == 0 else y_sb[:, lo : lo + 1]),
                        op0=ALU.add, op1=ALU.bypass,
                    )

                    nc.sync.dma_start(out=ox[sl, cs], in_=x_sb[:, cs])
                    nc.sync.dma_start(out=oy[sl, cs], in_=y_sb[:, cs])
                    nc.sync.dma_start(out=oyaw[sl, cs], in_=yaw_sb[:, cs])
                    nc.sync.dma_start(out=ospeed[sl, cs], in_=speed_sb[:, cs])

    nc.finalize()
    return nc


def _host_precompute(accel, steering):
    import ml_dtypes

    a = np.clip(accel.astype(np.float64), -1.0, 1.0)
    dv = DT * MAX_ACC * a
    c = np.concatenate([[0.0], np.cumsum(dv)[: H - 1]])
    st = np.clip(steering.astype(np.float64), -MAX_STEER, MAX_STEER)
    k = np.tan(st) / WHEELBASE * DT
    A = np.concatenate([[0.0], np.cumsum(k)[: H - 1]])
    Bv = np.concatenate([[0.0], np.cumsum(c * k)[: H - 1]])
    ones = np.ones(H)
    rhs_yaw = np.stack([A, Bv, ones]).astype(np.float32)
    rhs_t1 = np.stack(
        [-A * INV_2PI, -Bv * INV_2PI, -ones * INV_2PI, ones * MAGIC]
    ).astype(ml_dtypes.bfloat16)
    cdtrow = (DT * c).astype(np.float16)
    crow = c.astype(np.float32)
    return rhs_yaw, rhs_t1, cdtrow, crow


def _install_ntff_shim():
    """antenv.axon_hooks is absent in this image; recreate it so
    run_bass_kernel_spmd(trace=True) can reach the axon NTFF profiler."""
    import types

    import antenv

    if hasattr(antenv, "axon_hooks"):
        return
    mod = types.ModuleType("antenv.axon_hooks")
    holder = [None]
    mod.set_axon_ntff_profile_hook = lambda h: holder.__setitem__(0, h)
    mod.get_axon_ntff_profile_hook = lambda: holder[0]
    sys.modules["antenv.axon_hooks"] = mod
    antenv.axon_hooks = mod
    from trn_agent_boot.trn_boot import _ntff_profile_via_ctypes

    mod.set_axon_ntff_profile_hook(
        _ntff_profile_via_ctypes("/opt/axon/libaxon_pjrt.so")
    )


def run(start_x, start_y, start_yaw, start_speed, accel, steering, trace=False,
        tmpdir=None):
    import ml_dtypes

    if "nc" not in _CACHE:
        _CACHE["nc"] = _build()
    nc = _CACHE["nc"]
    if trace:
        _install_ntff_shim()

    start_x = np.asarray(start_x, dtype=np.float32)
    start_y = np.asarray(start_y, dtype=np.float32)
    start_yaw = np.asarray(start_yaw, dtype=np.float32)
    start_speed = np.asarray(start_speed, dtype=np.float32)
    rhs_yaw, rhs_t1, cdtrow, crow = _host_precompute(np.asarray(accel),
                                                     np.asarray(steering))

    in_maps = []
    ones = np.ones(BL, np.float32)
    for i in range(NCORES):
        sl = slice(i * BL, (i + 1) * BL)
        s0 = start_speed[sl]
        lhs3 = np.stack([s0, ones, start_yaw[sl]]).astype(np.float32)
        lhs4 = np.stack([s0, ones, start_yaw[sl], ones]).astype(
            ml_dtypes.bfloat16)
        cols = np.stack([start_x[sl], start_y[sl], s0, s0], axis=1).astype(
            np.float32)
        sdt = np.ascontiguousarray(
            (DT * s0.astype(np.float64)).astype(np.float16).reshape(NPT, P).T)
        in_maps.append({
            "rhs_yaw": rhs_yaw, "rhs_t1": rhs_t1, "cdtrow": cdtrow,
            "crow": crow,
            "lhs3r": np.ascontiguousarray(lhs3),
            "lhs4r": np.ascontiguousarray(lhs4),
            "cols": np.ascontiguousarray(cols),
            "sdth": np.ascontiguousarray(sdt),
        })

    res = run_bass_kernel_spmd(nc, in_maps, core_ids=list(range(NCORES)),
                               trace=trace, tmpdir=tmpdir)

    outs = []
    for key in ("ox", "oy", "oyaw", "ospeed"):
        full = np.concatenate([res.results[i][key] for i in range(NCORES)],
                              axis=0)
        outs.append(np.ascontiguousarray(full.T))
    return tuple(outs), res


def kernel(start_x, start_y, start_yaw, start_speed, accel, steering):
    outs, _ = run(start_x, start_y, start_yaw, start_speed, accel, steering)
    return outs


# revision 12
# speedup vs baseline: 1.0141x; 1.0141x over previous
"""Trainium2 Bass kernel for the kinematic bicycle-model rollout.

Strategy
--------
The recurrence is affine in the start state with batch-independent
coefficients, so the host precomputes (float64) the [H] vectors
    c[t]  = sum_{i<t} DT*MAX_ACC*clip(a_i)          (speed_t = s0 + c_t)
    A[t]  = sum_{i<t} k_i,  Bv[t] = sum_{i<t} c_i*k_i,
            k_i = tan(clip(s_i))/WHEELBASE*DT       (yaw_t = yaw0 + s0*A_t + Bv_t)
leaving on-chip:
    x_t = x0 + sum_{i<t} (DT*c_i + DT*s0) * cos(yaw_i)   (same for y with sin)

Per core (data-parallel over batch, 1024 rollouts/core), batch on the 128
SBUF partitions (8 tiles), time on the free dim (2048). The 33.5 MB/core
of output DMA (~108 us at ~320 GB/s) is the roofline; work is spread
across all four compute engines to approach it:
  - PE:   yaw = s0*A + Bv + yaw0 (K=3 fp32 matmul -> PSUM, ~69us) and
          speed = s0 + c (K=2 bf16 matmul, ~8us; bf16 keeps speed's
          rel err ~5e-3, well under the 2e-2 gate, and speed feeds
          nothing else)
  - ACT:  yaw and speed PSUM->SBUF copies; the magic-constant round
          t1a = u+MAGIC, t1b = MAGIC-t1a = -round(u) (u = yaw/2pi);
          sin = Sin(2pi*w) and cos = Sin(pi/2-2pi|w|) written as fp16
  - DVE:  w = u - round(u) (stt reading PSUM); vx = cdts0*cos (fp16
          tensor_tensor -> 2x packed mode); x/y prefix sums via
          tensor_tensor_scan (fp32 state, fp16 inputs)
  - GpSimd: cdts0 = DT*c + DT*s0 (stride-0 broadcast add) and
          vy = cdts0*sin, both fp16 tensor_tensor on the Q7 cores
Outputs are written batch-major [1024, 2048]; the host transposes to the
reference's [H, B] layout after gathering the 8 shards.
"""

import math
import sys

sys.path.insert(0, "/opt/trn_rl_repo")

import numpy as np

import concourse.bacc as bacc
import concourse.mybir as mybir
import concourse.tile as tile
from concourse.bass_utils import run_bass_kernel_spmd

# Model constants (match the reference nn.Module)
H = 2048
B = 8192
NCORES = 8
BL = B // NCORES          # batch per core
P = 128                   # SBUF partitions
NPT = BL // P             # batch tiles per core
DT = 0.05
WHEELBASE = 2.5
MAX_STEER = 0.5
MAX_ACC = 5000.0 / 1000.0

TWO_PI = 2.0 * math.pi
INV_2PI = 1.0 / TWO_PI
HALF_PI = 0.5 * math.pi
MAGIC = 12582912.0        # 1.5 * 2**23: x + MAGIC - MAGIC == round(x) in f32

F32 = mybir.dt.float32
F32R = mybir.dt.float32r
F16 = mybir.dt.float16
BF16 = mybir.dt.bfloat16
AFT = mybir.ActivationFunctionType
ALU = mybir.AluOpType

_CACHE = {}


def _build():
    nc = bacc.Bacc("TRN2", target_bir_lowering=False, debug=False)

    rhs_yaw = nc.declare_dram_parameter("rhs_yaw", [3, H], F32R, isOutput=False)
    rhs_t1 = nc.declare_dram_parameter("rhs_t1", [4, H], BF16, isOutput=False)
    cdtrow = nc.declare_dram_parameter("cdtrow", [H], F16, isOutput=False)
    crow = nc.declare_dram_parameter("crow", [H], F32, isOutput=False)
    lhs3r = nc.declare_dram_parameter("lhs3r", [3, BL], F32R, isOutput=False)
    lhs4r = nc.declare_dram_parameter("lhs4r", [4, BL], BF16, isOutput=False)
    cols = nc.declare_dram_parameter("cols", [BL, 4], F32, isOutput=False)
    sdth = nc.declare_dram_parameter("sdth", [P, NPT], F16, isOutput=False)
    ox = nc.declare_dram_parameter("ox", [BL, H], F32, isOutput=True)
    oy = nc.declare_dram_parameter("oy", [BL, H], F32, isOutput=True)
    oyaw = nc.declare_dram_parameter("oyaw", [BL, H], F32, isOutput=True)
    ospeed = nc.declare_dram_parameter("ospeed", [BL, H], F32, isOutput=True)

    with tile.TileContext(nc) as tc:
        with (
            tc.tile_pool(name="const", bufs=1) as constp,
            tc.tile_pool(name="io", bufs=2) as iop,
            tc.tile_pool(name="mid", bufs=2) as midp,
            tc.tile_pool(name="psum", bufs=2, space="PSUM") as psp,
            tc.tile_pool(name="cpool", bufs=1) as cdp,
        ):
            cdt_bc = constp.tile([P, H], F16)
            nc.sync.dma_start(out=cdt_bc[:],
                              in_=cdtrow[None, :].to_broadcast((P, H)))
            rhs_yaw_sb = constp.tile([3, H], F32R)
            nc.sync.dma_start(out=rhs_yaw_sb[:], in_=rhs_yaw[:])
            rhs_t1_sb = constp.tile([4, H], BF16)
            nc.sync.dma_start(out=rhs_t1_sb[:], in_=rhs_t1[:])
            c_bc = constp.tile([P, H], F32)
            nc.sync.dma_start(out=c_bc[:],
                              in_=crow[None, :].to_broadcast((P, H)))
            lhs3 = constp.tile([3, BL], F32R)
            nc.sync.dma_start(out=lhs3[:], in_=lhs3r[:])
            lhs4 = constp.tile([4, BL], BF16)
            nc.sync.dma_start(out=lhs4[:], in_=lhs4r[:])
            halfpi_col = constp.tile([P, 1], F32)
            nc.vector.memset(halfpi_col[:], HALF_PI)
            negmagic_col = constp.tile([P, 1], F32)
            nc.vector.memset(negmagic_col[:], -MAGIC)

            # all cdts0 tiles upfront: the Q7's slow semaphore path is
            # hidden behind the first tiles' compute
            sdt_all = constp.tile([P, NPT], F16)
            nc.sync.dma_start(out=sdt_all[:], in_=sdth[:])
            cdts0_all = []
            for pt in range(NPT):
                ct = cdp.tile([P, H], F16, tag=f"cdts0_{pt}")
                nc.gpsimd.tensor_tensor(
                    out=ct[:], in0=cdt_bc[:],
                    in1=sdt_all[:, pt : pt + 1].to_broadcast((P, H)),
                    op=ALU.add,
                )
                cdts0_all.append(ct)

            for pt in range(NPT):
                sl = slice(pt * P, (pt + 1) * P)
                colt = iop.tile([P, 4], F32, tag="colt")
                nc.sync.dma_start(out=colt[:], in_=cols[sl, :])
                x0_c = colt[:, 0:1]
                y0_c = colt[:, 1:2]
                s0_c = colt[:, 2:3]
                cdts0 = cdts0_all[pt]

                x_sb = iop.tile([P, H], F32, tag="x")
                y_sb = iop.tile([P, H], F32, tag="y")
                yaw_sb = iop.tile([P, H], F32, tag="yaw")
                speed_sb = iop.tile([P, H], F32, tag="speed")
                nc.scalar.activation(out=x_sb[:, 0:1], in_=x0_c, func=AFT.Copy)
                nc.scalar.activation(out=y_sb[:, 0:1], in_=y0_c, func=AFT.Copy)

                if True:
                    cs = slice(0, H)
                    HH = H
                    # front-end in half-width segments: yaw/t1 matmuls
                    # (fp32r / bf16-with-MAGIC-row-last), PSUM->SBUF copy,
                    # rnn = t1-MAGIC (Sterbenz), w = yaw/2pi - round(u)
                    w = midp.tile([P, HH], F32, tag="w")
                    for hf in range(2):
                        hs = slice(hf * (H // 2), (hf + 1) * (H // 2))
                        yaw_ps = psp.tile([P, H // 2], F32, tag="yawps")
                        t1_ps = psp.tile([P, H // 2], F32, tag="t1ps")
                        for j in range(H // 2 // 512):
                            js = slice(j * 512, (j + 1) * 512)
                            rs = slice(hf * (H // 2) + j * 512,
                                       hf * (H // 2) + (j + 1) * 512)
                            nc.tensor.matmul(yaw_ps[:, js], lhs3[:, sl],
                                             rhs_yaw_sb[:, rs])
                            nc.tensor.matmul(t1_ps[:, js], lhs4[:, sl],
                                             rhs_t1_sb[:, rs])
                        nc.scalar.activation(out=yaw_sb[:, hs], in_=yaw_ps[:],
                                             func=AFT.Copy)
                        rnn = midp.tile([P, H // 2], F32, tag="rnn")
                        nc.scalar.activation(out=rnn[:], in_=t1_ps[:],
                                             func=AFT.Identity,
                                             bias=negmagic_col[:])
                        nc.vector.scalar_tensor_tensor(
                            out=w[:, hs], in0=yaw_ps[:], scalar=INV_2PI,
                            in1=rnn[:], op0=ALU.mult, op1=ALU.add,
                        )
                    # speed = c + s0
                    nc.scalar.activation(out=speed_sb[:, cs], in_=c_bc[:, cs],
                                         func=AFT.Identity, bias=s0_c)

                    # sin/cos via the Sin LUT (valid on [-pi, pi]), fp16 out
                    sin_t = midp.tile([P, HH], F16, tag="sin")
                    nc.scalar.activation(out=sin_t[:], in_=w[:], func=AFT.Sin,
                                         scale=TWO_PI)
                    nc.scalar.activation(out=w[:], in_=w[:], func=AFT.Abs)
                    cos_t = midp.tile([P, HH], F16, tag="cos")
                    nc.scalar.activation(
                        out=cos_t[:], in_=w[:], func=AFT.Sin, scale=-TWO_PI,
                        bias=halfpi_col[:],
                    )

                    # vx on DVE (fp16 2x packed), vy on the Q7 cores
                    vx = midp.tile([P, HH], F16, tag="vx")
                    nc.vector.tensor_tensor(out=vx[:], in0=cdts0[:, cs],
                                            in1=cos_t[:], op=ALU.mult)
                    vy = midp.tile([P, HH], F16, tag="vy")
                    nc.vector.tensor_tensor(out=vy[:], in0=cdts0[:, cs],
                                            in1=sin_t[:], op=ALU.mult)

                    # x[1..H-1] = x0 + prefix(vx[0..H-2]); v[H-1] unused
                    nd = H - 1
                    nc.vector.tensor_tensor_scan(
                        out=x_sb[:, 1 : 1 + nd], data0=vx[:, 0:nd],
                        data1=vx[:, 0:nd], initial=x0_c,
                        op0=ALU.add, op1=ALU.bypass,
                    )
                    nc.vector.tensor_tensor_scan(
                        out=y_sb[:, 1 : 1 + nd], data0=vy[:, 0:nd],
                        data1=vy[:, 0:nd], initial=y0_c,
                        op0=ALU.add, op1=ALU.bypass,
                    )

                    nc.sync.dma_start(out=ox[sl, cs], in_=x_sb[:, cs])
                    nc.sync.dma_start(out=oy[sl, cs], in_=y_sb[:, cs])
                    nc.sync.dma_start(out=oyaw[sl, cs], in_=yaw_sb[:, cs])
                    nc.sync.dma_start(out=ospeed[sl, cs], in_=speed_sb[:, cs])

    nc.finalize()
    return nc


def _host_precompute(accel, steering):
    import ml_dtypes

    a = np.clip(accel.astype(np.float64), -1.0, 1.0)
    dv = DT * MAX_ACC * a
    c = np.concatenate([[0.0], np.cumsum(dv)[: H - 1]])
    st = np.clip(steering.astype(np.float64), -MAX_STEER, MAX_STEER)
    k = np.tan(st) / WHEELBASE * DT
    A = np.concatenate([[0.0], np.cumsum(k)[: H - 1]])
    Bv = np.concatenate([[0.0], np.cumsum(c * k)[: H - 1]])
    ones = np.ones(H)
    rhs_yaw = np.stack([A, Bv, ones]).astype(np.float32)
    rhs_t1 = np.stack(
        [-A * INV_2PI, -Bv * INV_2PI, -ones * INV_2PI, ones * MAGIC]
    ).astype(ml_dtypes.bfloat16)
    cdtrow = (DT * c).astype(np.float16)
    crow = c.astype(np.float32)
    return rhs_yaw, rhs_t1, cdtrow, crow


def _install_ntff_shim():
    """antenv.axon_hooks is absent in this image; recreate it so
    run_bass_kernel_spmd(trace=True) can reach the axon NTFF profiler."""
    import types

    import antenv

    if hasattr(antenv, "axon_hooks"):
        return
    mod = types.ModuleType("antenv.axon_hooks")
    holder = [None]
    mod.set_axon_ntff_profile_hook = lambda h: holder.__setitem__(0, h)
    mod.get_axon_ntff_profile_hook = lambda: holder[0]
    sys.modules["antenv.axon_hooks"] = mod
    antenv.axon_hooks = mod
    from trn_agent_boot.trn_boot import _ntff_profile_via_ctypes

    mod.set_axon_ntff_profile_hook(
        _ntff_profile_via_ctypes("/opt/axon/libaxon_pjrt.so")
    )


def run(start_x, start_y, start_yaw, start_speed, accel, steering, trace=False,
        tmpdir=None):
    import ml_dtypes

    if "nc" not in _CACHE:
        _CACHE["nc"] = _build()
    nc = _CACHE["nc"]
    if trace:
        _install_ntff_shim()

    start_x = np.asarray(start_x, dtype=np.float32)
    start_y = np.asarray(start_y, dtype=np.float32)
    start_yaw = np.asarray(start_yaw, dtype=np.float32)
    start_speed = np.asarray(start_speed, dtype=np.float32)
    rhs_yaw, rhs_t1, cdtrow, crow = _host_precompute(np.asarray(accel),
                                                     np.asarray(steering))

    in_maps = []
    ones = np.ones(BL, np.float32)
    for i in range(NCORES):
        sl = slice(i * BL, (i + 1) * BL)
        s0 = start_speed[sl]
        lhs3 = np.stack([s0, ones, start_yaw[sl]]).astype(np.float32)
        lhs4 = np.stack([s0, ones, start_yaw[sl], ones]).astype(
            ml_dtypes.bfloat16)
        cols = np.stack([start_x[sl], start_y[sl], s0, s0], axis=1).astype(
            np.float32)
        sdt = np.ascontiguousarray(
            (DT * s0.astype(np.float64)).astype(np.float16).reshape(NPT, P).T)
        in_maps.append({
            "rhs_yaw": rhs_yaw, "rhs_t1": rhs_t1, "cdtrow": cdtrow,
            "crow": crow,
            "lhs3r": np.ascontiguousarray(lhs3),
            "lhs4r": np.ascontiguousarray(lhs4),
            "cols": np.ascontiguousarray(cols),
            "sdth": np.ascontiguousarray(sdt),
        })

    res = run_bass_kernel_spmd(nc, in_maps, core_ids=list(range(NCORES)),
                               trace=trace, tmpdir=tmpdir)

    outs = []
    for key in ("ox", "oy", "oyaw", "ospeed"):
        full = np.concatenate([res.results[i][key] for i in range(NCORES)],
                              axis=0)
        outs.append(np.ascontiguousarray(full.T))
    return tuple(outs), res


def kernel(start_x, start_y, start_yaw, start_speed, accel, steering):
    outs, _ = run(start_x, start_y, start_yaw, start_speed, accel, steering)
    return outs
